# revision 1
# baseline (speedup 1.0000x reference)
"""Trainium2 Bass kernel for the DGCNN-style message-passing block.

Math (per batch b, data-parallel over 8 cores):
    proj = x @ Wp^T
    m[i] = max_k proj[knn[i,k]]           (edge maxpool: max_k(f_j - f_i) = m_i - proj_i)
    x1 = BN_l((m - proj) @ Wl^T);  x2 = BN_g(x @ Wg^T)
    h  = BN_1((x1+x2) @ W1^T + b1); a = sigmoid(BN_2(h @ W2^T + b2))
    out = BN_f(a*x1 + (1-a)*x2)

All BNs are inference-affine and fold into the weights host-side, and proj
composes into the local branch (x1 = m@Wl'^T - x@(Wl'Wp)^T + t_l), giving:
    f  = m@Wmf^T + x@Wxf^T + tf          (= x1+x2, feeds h)
    d' = m@Wmd^T + x@Wxd^T + td          (= s_f*(x1-x2))
    v  = x@Wxv^T + tv                    (= BN_f(x2))
    h  = f@W1'^T + t1;  a = sigmoid(h@W2'^T + t2)
    out = v + a*d'

Everything runs in bf16 (PSUM accumulation stays f32): x^T, the 8 packed
weight matrices, proj, the gather stream, and the output (the host casts back
to f32).  On-chip layout is feature-major; the KNN max-gather runs per
node-group (three 1024-node groups + two 512-node tail groups for a shorter
exposed tail) as 16 `dma_gather` custom ops over 4 SWDGE queues from a bf16
node-major copy of proj in DRAM, followed by a DVE max-accumulate pair of
chains; m^T comes back via PE transposes (evacuated on Act).  The schedule is
software-pipelined one group deep (gathers+maxes of group g issue before the
matmul passes of g-1), pass PSUM evacuations run on Act (DVE is near
saturation pacing the max chains against the gather stream).  Note the
512-node tail groups make their gather stretch Pool-desc-gen-paced (994ns
fixed cost per gather > 728ns DMA), costing ~8us of stream; this is a
measured net win vs. uniform 1024 groups because it buys a ~10us shorter
exposed tail.  The proj head is pipelined: 2 node-tiles per
PSUM buffer x3 buffers, Act/DVE alternating evacuation, 16-chunk DRAM write,
and deferred weight/bias/index loads, so the first gathers start ~37us
earlier than the original monolithic head.
"""

import numpy as np
import ml_dtypes

import concourse.bass as bass
import concourse.mybir as mybir
import concourse.tile as tile
from concourse import bacc
from concourse.bass_utils import run_bass_kernel_spmd
from concourse.masks import make_identity

F32 = mybir.dt.float32
BF16 = mybir.dt.bfloat16
I16 = mybir.dt.int16

B, N, K, C = 8, 4096, 16, 256
P = 128
NT = N // P          # 32 node tiles / stripes
CK = C // P          # 2 channel chunks
NQ = 4               # node quarters
QN = N // NQ         # 1024 nodes per quarter
HN = 512             # pass-slice width
EPS = 1e-5

# node groups: three 1024-wide + two 512-wide.  The 512-groups' higher
# per-gather desc-gen overhead (994ns fixed) makes their gather stretch
# Pool-paced, costing ~8us of stream vs uniform 1024 groups — but measured
# end-to-end this still wins by ~2-9us: every uniform-group variant tried
# (ws=256/512 tail slicing, outs on DVE/Pool) loses more in the exposed tail.
GROUPS = [(0, 1024), (1024, 1024), (2048, 1024), (3072, 512), (3584, 512)]
KCOLS = K * sum(sz // 16 for _, sz in GROUPS)

AF = mybir.ActivationFunctionType


def build_bass(n_cores: int = 8, reps: int = 1):
    nc = bacc.Bacc(
        "TRN2",
        target_bir_lowering=False,
        debug=False,
        enable_asserts=False,
        num_devices=n_cores,
        num_swdge_queues=4,
    )

    xT = nc.dram_tensor("xT", [C, N], BF16, kind="ExternalInput").ap()
    knn_i = nc.dram_tensor("knn_i", [P, KCOLS], I16, kind="ExternalInput").ap()
    # packed weights: [128, (w, kc, 256)] bf16; order: wpT,wxf,wxd,wxv,wmf,wmd,w1,w2
    wb = nc.dram_tensor("wb", [P, 8 * CK * C], BF16, kind="ExternalInput").ap()
    bias = nc.dram_tensor("bias", [P, 10], F32, kind="ExternalInput").ap()
    outT = nc.dram_tensor("outT", [C, N], BF16, kind="ExternalOutput").ap()

    with tile.TileContext(nc) as tc:
        for _ in range(reps):
            kernel_body(tc, xT, knn_i, wb, bias, outT)
    nc.compile()
    return nc


def kernel_body(tc, xT, knn_i, wb, bias, outT):
    nc = tc.nc

    with (
        tc.tile_pool(name="const", bufs=1) as cpool,
        tc.tile_pool(name="projp", bufs=1) as projp,
        tc.tile_pool(name="gat", bufs=1) as gat,
        tc.tile_pool(name="mt", bufs=3) as mtp,
        tc.tile_pool(name="units", bufs=2) as up,
        tc.tile_pool(name="outp", bufs=3) as outp,
        tc.tile_pool(name="psA", bufs=4, space="PSUM") as psA,
        tc.tile_pool(name="psB", bufs=1, space="PSUM") as psB,
        tc.tile_pool(name="psNP", bufs=3, space="PSUM") as psNP,
        tc.tile_pool(name="dram", bufs=1, space="DRAM") as dpool,
    ):
        # ---- load order: wpT (proj weights), group-0 indices, xT chunks; the
        # rest of the weights / bias / later-group indices are issued after
        # the proj writes so they don't delay the first gathers.
        wb_sb = cpool.tile([P, 8 * CK * C], BF16)
        nc.sync.dma_start(wb_sb[:, :CK * C], wb[:, :CK * C])          # wpT first

        # per-group index loads; host supplies final SBUF layout [128, cols]
        kidx_all = cpool.tile([P, KCOLS], I16)
        goff = []  # starting col of each group's block
        off = 0
        for _, sz in GROUPS:
            goff.append(off)
            off += K * (sz // 16)  # total cols invariant to gather merging

        def load_kidx(g):
            lo = goff[g]
            hi = goff[g + 1] if g + 1 < len(GROUPS) else KCOLS
            nc.sync.dma_start(kidx_all[:, lo:hi], knn_i[:, lo:hi])

        kidx = {}
        for g, (_, sz) in enumerate(GROUPS):
            cols = sz // 16
            for k in range(K):
                lo = goff[g] + k * cols
                kidx[(g, k)] = kidx_all[:, lo:lo + cols]

        xt = []  # xt[kc][cc]: [128, QN] chunk of x^T
        for cc in range(NQ):
            for kc in range(CK):
                if cc == 0:
                    xt.append([])
                t = cpool.tile([P, QN], BF16, name=f"xt{kc}_{cc}")
                nc.sync.dma_start(t[:], xT[kc * P:(kc + 1) * P, cc * QN:(cc + 1) * QN])
                xt[kc].append(t)

        load_kidx(0)

        def xt_slice(kc, n0, sl):
            """x^T slice for absolute node range [n0+sl.start, n0+sl.stop)."""
            cc, off = n0 // QN, n0 % QN
            return xt[kc][cc][:, off + sl.start:off + sl.stop]

        def wslice(w_i):
            return [wb_sb[:, (w_i * CK + kc) * C:(w_i * CK + kc + 1) * C] for kc in range(CK)]

        wpT_sb = wslice(0)
        wxf_sb = wslice(1)
        wxd_sb = wslice(2)
        wxv_sb = wslice(3)
        wmf_sb = wslice(4)
        wmd_sb = wslice(5)
        w1_sb = wslice(6)
        w2_sb = wslice(7)

        bias_sb = cpool.tile([P, 10], F32)

        ident = cpool.tile([P, P], BF16)
        make_identity(nc, ident[:])

        # ---- phase 1: proj, node-major bf16 -> DRAM (pipelined, 4-chunk write)
        # proj_dram row r = p*NT + s holds node n = s*P + p (p-major permutation,
        # host permutes the knn indices to match). 2 node-tiles share a PSUM buf;
        # evacuation alternates Act/DVE; the DRAM write goes out in 4 chunks.
        proj_sb = projp.tile([P, NT, C], BF16)
        proj_dram = dpool.tile([N, C], BF16)
        proj_dram_v = proj_dram[:].rearrange("(p s) c -> p (s c)", p=P)
        WCH = NT // 4  # stripes per write chunk
        for tp in range(NT // 2):
            ps = psNP.tile([P, 2, C], F32, name="ps_np", tag="ps_np")
            for j in range(2):
                t = 2 * tp + j
                cc, tl = t // (QN // P), t % (QN // P)
                nc.tensor.matmul(ps[:, j, :], lhsT=xt[0][cc][:, tl * P:(tl + 1) * P],
                                 rhs=wpT_sb[0], start=True, stop=False,
                                 skip_group_check=True)
                nc.tensor.matmul(ps[:, j, :], lhsT=xt[1][cc][:, tl * P:(tl + 1) * P],
                                 rhs=wpT_sb[1], start=False, stop=True,
                                 skip_group_check=True)
            if tp % 2 == 0:
                nc.scalar.activation(proj_sb[:, 2 * tp:2 * tp + 2, :], ps[:], AF.Copy)
            else:
                nc.vector.tensor_copy(proj_sb[:, 2 * tp:2 * tp + 2, :], ps[:])
            if (tp + 1) % 2 == 0:
                wc = (tp + 1) // 2 - 1  # 8 write chunks of 4 stripes
                nc.sync.dma_start(
                    proj_dram_v[:, wc * 4 * C:(wc + 1) * 4 * C],
                    proj_sb[:, wc * 4:(wc + 1) * 4, :])

        # ---- phases 2-4: software-pipelined per node-group ----
        # stage A(g): 16 x dma_gather + DVE max chains + PE transpose
        # stage B(g): f/d/h/a matmul passes + out, in 512-node slices

        def evac(engine, out_ap, ps, bcol_mc, func=AF.Identity):
            """PSUM -> SBUF with per-channel bias; Act or DVE."""
            if engine == "act" or func != AF.Identity:
                nc.scalar.activation(out_ap, ps[:], func,
                                     bias=bias_sb[:, bcol_mc:bcol_mc + 1],
                                     scale=1.0)
            else:
                nc.vector.tensor_scalar(
                    out=out_ap, in0=ps[:],
                    scalar1=bias_sb[:, bcol_mc:bcol_mc + 1], scalar2=None,
                    op0=mybir.AluOpType.add)

        def mx_pass(n0, mt, sl, out_sb, wm_sb, wx_sb, bcol, mc, eng="act",
                    func=AF.Identity):
            """psum = [m-part] + [x-part] over a node slice; evac w/ bias."""
            ps = psA.tile([P, sl.stop - sl.start], F32, name="ps_fp", tag="ps_fp")
            first = True
            if wm_sb is not None:
                for kc in range(CK):
                    nc.tensor.matmul(
                        ps[:], lhsT=wm_sb[kc][:, mc * P:(mc + 1) * P],
                        rhs=mt[:, kc, sl], start=first and kc == 0,
                        stop=False, skip_group_check=True)
                first = False
            for kc in range(CK):
                last = kc == CK - 1
                nc.tensor.matmul(
                    ps[:], lhsT=wx_sb[kc][:, mc * P:(mc + 1) * P],
                    rhs=xt_slice(kc, n0, sl),
                    start=first and kc == 0, stop=last,
                    skip_group_check=True)
            evac(eng, out_sb[:, mc, sl], ps, bcol + mc, func)

        def hx_pass(w_sb, in_sb, sl, out_sb, bcol, mc, eng="act",
                    func=AF.Identity):
            ps = psA.tile([P, sl.stop - sl.start], F32, name="ps_fp", tag="ps_fp")
            for kc in range(CK):
                nc.tensor.matmul(
                    ps[:], lhsT=w_sb[kc][:, mc * P:(mc + 1) * P],
                    rhs=in_sb[:, kc, sl],
                    start=kc == 0, stop=kc == CK - 1,
                    skip_group_check=True)
            evac(eng, out_sb[:, mc, sl], ps, bcol + mc, func)

        # deferred loads (slot into the gather stream, needed ~25us in)
        nc.sync.dma_start(wb_sb[:, CK * C:], wb[:, CK * C:])
        nc.sync.dma_start(bias_sb[:], bias[:])
        # groups 1..4 indices are contiguous columns: one DMA, fewer per-op
        # latencies in the pre-gather window
        nc.sync.dma_start(kidx_all[:, goff[1]:], knn_i[:, goff[1]:])

        vg = [None] * len(GROUPS)

        def stage_a(g):
            n0, sz = GROUPS[g]
            gt = sz // P  # node stripes in this group
            sls = [slice(i * HN, (i + 1) * HN) for i in range(sz // HN)]

            # v = x@wxv (x-only; hoisted here so stage_b's tail is shorter)
            v_sb = up.tile([P, CK, sz], BF16, name="v_sb", tag="v")
            vg[g] = v_sb
            for sl in sls:
                for mc in range(CK):
                    mx_pass(n0, None, sl, v_sb, None, wxv_sb, 4, mc, eng="act")

            gk = [None] * K
            for k in range(K):
                small = sz < QN
                gtl = gat.tile([P, gt, C], BF16, name=f"g_{g}_{k}",
                               tag="gks" if small else "gk",
                               bufs=16 if small else 8)
                nc.gpsimd.dma_gather(
                    out_ap=gtl[:],
                    in_ap=proj_dram[:],
                    idxs_ap=kidx[(g, k)],
                    num_idxs=sz,
                    num_idxs_reg=sz,
                    elem_size=C,
                    queue_num=k % 4,
                )
                gk[k] = gtl[:]
            # two max chains + combine.  Mid-stream groups: sequential
            # halves, both on DVE.  Last group: even/odd chains on DVE/Pool
            # (Pool is done with desc-gen by then), so only ~2 ops trail the
            # last gather.
            accA = gat.tile([P, gt, C], BF16, name=f"accA{g}", tag="accA", bufs=3)
            accB = gat.tile([P, gt, C], BF16, name=f"accB{g}", tag="accB", bufs=3)
            nc.vector.tensor_tensor(out=accA[:], in0=gk[0], in1=gk[1],
                                    op=mybir.AluOpType.max)
            for k in range(2, K // 2):
                nc.vector.tensor_tensor(out=accA[:], in0=accA[:], in1=gk[k],
                                        op=mybir.AluOpType.max)
            nc.vector.tensor_tensor(out=accB[:], in0=gk[K // 2],
                                    in1=gk[K // 2 + 1],
                                    op=mybir.AluOpType.max)
            for k in range(K // 2 + 2, K):
                nc.vector.tensor_tensor(out=accB[:], in0=accB[:], in1=gk[k],
                                        op=mybir.AluOpType.max)
            mt = mtp.tile([P, CK, sz], BF16, name="mt", tag="mt")
            if g == len(GROUPS) - 1:
                # last group: combine + transpose + evac in 2-stripe (256
                # node) sub-slices so stage_b starts ~1us after the last
                # gather instead of waiting for the full-group combine;
                # kc0 evacs on Act, kc1 on DVE so sub-slices drain in parallel
                for sub in range(gt // 2):
                    j0 = sub * 2
                    nc.vector.tensor_tensor(
                        out=accA[:, j0:j0 + 2, :], in0=accA[:, j0:j0 + 2, :],
                        in1=accB[:, j0:j0 + 2, :], op=mybir.AluOpType.max)
                    for kc in range(CK):
                        pst = psB.tile([P, 2 * P], BF16, name="pst", tag="pst")
                        for j in (0, 1):
                            nc.tensor.transpose(
                                pst[:, j * P:(j + 1) * P],
                                accA[:, j0 + j, kc * P:(kc + 1) * P], ident[:])
                        nc.scalar.activation(
                            mt[:, kc, j0 * P:(j0 + 2) * P], pst[:], AF.Copy)
                return mt
            nc.vector.tensor_tensor(out=accA[:], in0=accA[:], in1=accB[:],
                                    op=mybir.AluOpType.max)
            acc = accA

            # m^T via PE transpose: [node, c] -> [c, node]; evacs on Act
            for kc in range(CK):
                pst = psB.tile([P, sz], BF16, name="pst", tag="pst")
                for j in range(gt):
                    nc.tensor.transpose(pst[:, j * P:(j + 1) * P],
                                        acc[:, j, kc * P:(kc + 1) * P], ident[:])
                nc.scalar.activation(mt[:, kc, :], pst[:], AF.Copy)
            return mt

        def stage_b(g, mt, split_evac=False, ws=HN):
            n0, sz = GROUPS[g]
            eng2 = "dve" if split_evac else "act"
            sls = [slice(i * ws, (i + 1) * ws) for i in range(sz // ws)]
            f_sb = up.tile([P, CK, sz], BF16, name="f_sb", tag="f")
            d_sb = up.tile([P, CK, sz], BF16, name="d_sb", tag="d")
            h_sb = up.tile([P, CK, sz], BF16, name="h_sb", tag="h")
            a_sb = up.tile([P, CK, sz], BF16, name="a_sb", tag="a")
            v_sb = vg[g]
            # PE order f,f,h,h,d,d,a,a keeps PE busy across evac waits
            for sl in sls:
                for mc in range(CK):
                    mx_pass(n0, mt, sl, f_sb, wmf_sb, wxf_sb, 0, mc,
                            eng="act" if mc == 0 else eng2)
            for sl in sls:
                for mc in range(CK):
                    hx_pass(w1_sb, f_sb, sl, h_sb, 6, mc,
                            eng="act" if mc == 0 else eng2)
            for sl in sls:
                for mc in range(CK):
                    mx_pass(n0, mt, sl, d_sb, wmd_sb, wxd_sb, 2, mc,
                            eng="act" if mc == 0 else eng2)
            for sl in sls:
                for mc in range(CK):
                    hx_pass(w2_sb, h_sb, sl, a_sb, 8, mc, eng="act",
                            func=AF.Sigmoid)
            # group len-2's outs run while the last group's max chains own
            # DVE; Pool is done with desc-gen by then, so use it there
            oeng = nc.gpsimd if g == len(GROUPS) - 2 else nc.vector
            for sl in sls:
                # out = v + a*d'  (bf16, op pair + one DMA)
                ot = outp.tile([P, CK, sl.stop - sl.start], BF16,
                               name="ot", tag="ot")
                oeng.tensor_tensor(out=ot[:], in0=a_sb[:, :, sl],
                                   in1=d_sb[:, :, sl],
                                   op=mybir.AluOpType.mult)
                oeng.tensor_tensor(out=ot[:], in0=ot[:], in1=v_sb[:, :, sl],
                                   op=mybir.AluOpType.add)
                nn = n0 + sl.start
                nc.sync.dma_start(
                    outT[:, nn:nn + (sl.stop - sl.start)].rearrange(
                        "(k p) n -> p k n", p=P),
                    ot[:])

        mt_prev = stage_a(0)
        for g in range(1, len(GROUPS)):
            mt_g = stage_a(g)
            stage_b(g - 1, mt_prev)
            mt_prev = mt_g
        stage_b(len(GROUPS) - 1, mt_prev, split_evac=True, ws=256)


# ---------------- host side ----------------

def _fold(proj_W, local_W, glob_W, aff_W1, aff_b1, aff_W2, aff_b2,
          bn_local, bn_glob, bn_aff1, bn_aff2, bn_final):
    f32 = np.float32

    def bn_st(p):
        p = np.asarray(p, f32)
        g, b, m, v = p
        s = g / np.sqrt(v + EPS)
        return s.astype(f32), (b - m * s).astype(f32)

    Wp = np.asarray(proj_W, f32)
    s_l, t_l = bn_st(bn_local)
    s_g, t_g = bn_st(bn_glob)
    s_1, t_1 = bn_st(bn_aff1)
    s_2, t_2 = bn_st(bn_aff2)
    s_f, t_f = bn_st(bn_final)

    Wlp = s_l[:, None] * np.asarray(local_W, f32)
    Wgp = s_g[:, None] * np.asarray(glob_W, f32)
    Wlproj = (Wlp @ Wp).astype(f32)

    def pack(ws, dt):
        # ws: list of [C, C] W^T arrays -> [128, n*CK*C]: block (w_i, kc) = W^T[kc*128:(kc+1)*128, :]
        P_, CK_ = 128, 2
        out = np.zeros((P_, len(ws) * CK_ * 256), dt)
        for w_i, m in enumerate(ws):
            for kc in range(CK_):
                out[:, (w_i * CK_ + kc) * 256:(w_i * CK_ + kc + 1) * 256] = m[kc * P_:(kc + 1) * P_, :].astype(dt)
        return out

    w = {}
    wpT = np.ascontiguousarray(Wp.T)
    wxf = np.ascontiguousarray((Wgp - Wlproj).T)
    wxd = np.ascontiguousarray((-s_f[:, None] * (Wlproj + Wgp)).T)
    wxv = np.ascontiguousarray((s_f[:, None] * Wgp).T)
    wmf = np.ascontiguousarray(Wlp.T)
    wmd = np.ascontiguousarray((s_f[:, None] * Wlp).T)
    w1 = np.ascontiguousarray((s_1[:, None] * np.asarray(aff_W1, f32)).T)
    w2 = np.ascontiguousarray((s_2[:, None] * np.asarray(aff_W2, f32)).T)
    w["wb"] = pack([wpT, wxf, wxd, wxv, wmf, wmd, w1, w2], ml_dtypes.bfloat16)

    tf = t_l + t_g
    td = s_f * (t_l - t_g)
    tv = s_f * t_g + t_f
    t1 = s_1 * np.asarray(aff_b1, f32) + t_1
    t2 = s_2 * np.asarray(aff_b2, f32) + t_2
    # bias[p, 2*j + mc] = coeff_j[mc*128 + p]
    bias = np.zeros((P, 10), f32)
    for j, tt in enumerate((tf, td, tv, t1, t2)):
        for mc in range(CK):
            bias[:, 2 * j + mc] = tt[mc * P:(mc + 1) * P]
    w["bias"] = bias
    return w


_NC_CACHE = {}


def _get_nc():
    if "nc" not in _NC_CACHE:
        _NC_CACHE["nc"] = build_bass(B)
    return _NC_CACHE["nc"]


def kernel(**inputs) -> np.ndarray:
    x = np.ascontiguousarray(np.asarray(inputs["x"], np.float32))      # [B,N,C]
    knn = np.asarray(inputs["knn"]).astype(np.int64)                   # [B,N,K]
    w = _fold(
        inputs["proj_W"], inputs["local_W"], inputs["glob_W"],
        inputs["aff_W1"], inputs["aff_b1"], inputs["aff_W2"], inputs["aff_b2"],
        inputs["bn_local"], inputs["bn_glob"], inputs["bn_aff1"],
        inputs["bn_aff2"], inputs["bn_final"],
    )

    # proj_dram row permutation: node n lives at row (n%128)*32 + n//128
    r = ((knn % P) * NT + knn // P).astype(np.int16)                   # [B,N,K]
    # dma_gather wrapped layout, group-major [B, 128, KCOLS]: per (group, k)
    # a [128, sz/16] block where flat i -> [i%16, i//16], replicated 8x
    # across partition groups
    blocks = []
    for n0, sz in GROUPS:
        ids = r[:, n0:n0 + sz, :]                                  # [B, sz, K]
        wr = ids.reshape(B, sz // 16, 16, K).transpose(0, 3, 2, 1)  # [B,K,16,c]
        wr = np.tile(wr, (1, 1, 8, 1))                             # [B,K,128,c]
        blocks.append(wr.transpose(0, 2, 1, 3).reshape(B, P, -1))  # [B,128,K*c]
    ridx = np.concatenate(blocks, axis=2).astype(np.int16)         # [B,128,KCOLS]

    nc = _get_nc()
    in_maps = []
    for b in range(B):
        m = {"xT": np.ascontiguousarray(x[b].T).astype(ml_dtypes.bfloat16),
             "knn_i": np.ascontiguousarray(ridx[b])}
        for k2, v in w.items():
            m[k2] = v
        in_maps.append(m)

    res = run_bass_kernel_spmd(nc, in_maps, core_ids=list(range(B)))
    out = np.stack([res.results[b]["outT"].astype(np.float32).T for b in range(B)])
    return out.astype(np.float32)


if __name__ == "__main__":
    nc = build_bass(1)
    print("built OK")

